# revision 5
# baseline (speedup 1.0000x reference)
"""Trainium2 Bass kernel for nn_BasicBlock (MoE-combined residual conv block).

  out = relu(bn2(conv3x3(relu(bn1(conv3x3(x, w1e))), w2e)) + x)
  w{1,2}e = sum_e alpha[e] * w{1,2}[e]   (host-side: linear in weights)

Strategy (per NeuronCore, data-parallel over batch: 32 imgs -> 4 per core x 8):
  - x is shipped from the host already zero-padded to [64, 114, 114] and cast
    to fp16, flattened to [64, 12996]. Each image's conv input lives in SBUF
    as TWO 128-partition fp16 tiles loaded straight from HBM (no on-device
    casts, memsets or shuffles):
      T1 = [A ; A>>1row]   (upper = same HBM buffer at flat offset +114)
      T2 = [A ; A<<1col]   (upper = flat offset +1)
  - A matmul pass contracts 128 partitions = 2 planes x 64ch; with row
    offsets of T1/T2 a 3x3 conv needs only FIVE passes per output chunk
    (PE cost is N_free cycles regardless of K, so pass count is everything):
      3x T2@rowoff d: taps (d,0)+(d,1)   [K=128]
      1x T1@rowoff 0, coloff 2: taps (0,2)+(1,2)   [K=128]
      1x T1 upper@rowoff 1, coloff 2: tap (2,2)    [K=64]
  - conv2 keeps the 6-pass structure on M1 = [M ; M>>1row] (3 pair passes +
    3 singles) so the mid plane needs no extra DMA copies: ACT evicts the M
    half (relu+bn1 fused, scale folded into w1), DVE evicts the M>>1 half --
    split across engines so neither outruns the PE group time.
  - conv2 epilogue on DVE: residual add (fp16 x from T1 lower) straight out
    of PSUM, then relu (+bn2 bias; bn2 scale folded into w2) into a per-group
    fp16 tile, one DMA per 16-row group to HBM. Host upcasts to fp32.
"""

import numpy as np

import concourse.mybir as mybir
import concourse.tile as tile
from concourse import bacc
from concourse.bass_utils import run_bass_kernel_spmd

F32 = mybir.dt.float32
F16 = mybir.dt.float16
AF = mybir.ActivationFunctionType
ALU = mybir.AluOpType

EPS = 1e-5
N_CORES = 8
C = 64   # channels (in == out)
R = 4    # output rows per PSUM chunk
G = 4    # chunks per weight-stationary group


def build_nc(B, H, W):
    """Bass program: B images of [64, H, W] per core, pre-padded fp16 input."""
    Hp, Wp = H + 2, W + 2
    FLAT = Hp * Wp
    N = R * W                     # psum free size per chunk
    nchunks = H // R
    assert H % R == 0 and nchunks % G == 0
    ngroups = nchunks // G

    nc = bacc.Bacc("TRN2", target_bir_lowering=False, debug=False,
                   enable_asserts=False, num_devices=N_CORES)

    xin = nc.dram_tensor("xin", [B, C, FLAT], F16, kind="ExternalInput").ap()
    w1p2_d = nc.dram_tensor("w1p2", [128, 3 * 128], F16, kind="ExternalInput").ap()
    w1p1_d = nc.dram_tensor("w1p1", [128, 128], F16, kind="ExternalInput").ap()
    w1s_d = nc.dram_tensor("w1s", [64, 128], F16, kind="ExternalInput").ap()
    w2p_d = nc.dram_tensor("w2p", [128, 3 * 64], F16, kind="ExternalInput").ap()
    w2s_d = nc.dram_tensor("w2s", [64, 3 * 64], F16, kind="ExternalInput").ap()
    b1_d = nc.dram_tensor("b1", [128, 1], F32, kind="ExternalInput").ap()
    b2_d = nc.dram_tensor("b2", [64, 1], F32, kind="ExternalInput").ap()
    yout = nc.dram_tensor("yout", [B, C, H * W], F16, kind="ExternalOutput").ap()

    with tile.TileContext(nc) as tc:
        with (
            tc.tile_pool(name="wpool", bufs=1) as wpool,
            tc.tile_pool(name="xpool", bufs=2) as xpool,
            tc.tile_pool(name="mpool", bufs=1) as mpool,
            tc.tile_pool(name="pspool", bufs=8, space="PSUM") as pspool,
            tc.tile_pool(name="upool", bufs=4) as upool,
            tc.tile_pool(name="opool", bufs=3) as opool,
        ):
            w1p2 = wpool.tile([128, 3 * 128], F16)
            w1p1 = wpool.tile([128, 128], F16)
            w1s = wpool.tile([128, 128], F16)   # content in parts 64-127
            w2p = wpool.tile([128, 3 * 64], F16)
            w2s = wpool.tile([128, 3 * 64], F16)  # content in parts 64-127
            b1t = wpool.tile([128, 1], F32)
            b2t = wpool.tile([64, 1], F32)
            nc.sync.dma_start(w1p2[:, :], w1p2_d[:, :])
            nc.sync.dma_start(w1p1[:, :], w1p1_d[:, :])
            nc.sync.dma_start(w1s[64:128, :], w1s_d[:, :])
            nc.sync.dma_start(w2p[:, :], w2p_d[:, :])
            nc.sync.dma_start(w2s[64:128, :], w2s_d[:, :])
            nc.sync.dma_start(b1t[:, :], b1_d[:, :])
            nc.sync.dma_start(b2t[:, :], b2_d[:, :])

            def make_x_tiles(img):
                t1 = xpool.tile([128, FLAT], F16, tag="t1", name=f"t1_{img}")
                t2 = xpool.tile([128, FLAT], F16, tag="t2", name=f"t2_{img}")
                return t1, t2

            def load_t2(eng, t2, img, bands):
                src = xin[img]
                for r0, r1 in bands:
                    a, b = r0 * Wp, r1 * Wp
                    eng.dma_start(t2[0:64, a:b], src[:, a:b])
                    bb = min(b, FLAT - 1)
                    eng.dma_start(t2[64:128, a:bb], src[:, a + 1:bb + 1])

            def load_t1(eng, t1, img, bands):
                src = xin[img]
                for r0, r1 in bands:
                    a, b = r0 * Wp, r1 * Wp
                    eng.dma_start(t1[0:64, a:b], src[:, a:b])
                    bb = min(b, FLAT - Wp)
                    if bb > a:
                        eng.dma_start(t1[64:128, a:bb], src[:, a + Wp:bb + Wp])

            # image 0: tiny first band on the (otherwise idle) sync queue so
            # the first matmul group starts ASAP; the rest progressively on
            # two separate queues so transfers run in parallel with compute.
            cur = make_x_tiles(0)
            load_t2(nc.sync, cur[1], 0, [(0, 6)])
            load_t1(nc.sync, cur[0], 0, [(0, 6)])
            rest = [(6, 16), (16, 32), (32, 56), (56, 86), (86, Hp)]
            load_t2(nc.scalar, cur[1], 0, rest)
            load_t1(nc.gpsimd, cur[0], 0, rest)

            for img in range(B):
                t1f, t2f = cur
                t1 = t1f[:, :].rearrange("p (h w) -> p h w", w=Wp)
                t2 = t2f[:, :].rearrange("p (h w) -> p h w", w=Wp)

                # ---- mid plane M1 = [M ; M>>1], written by ACT/DVE ----
                mt = mpool.tile([128, FLAT], F16, tag="mt", name=f"mt_{img}")
                mr = mt[:, :].rearrange("p (h w) -> p h w", w=Wp)
                nc.gpsimd.memset(mr[0:64, 0, :], 0.0)
                nc.gpsimd.memset(mr[0:64, Hp - 1, :], 0.0)
                nc.gpsimd.memset(mr[64:128, H, :], 0.0)
                nc.gpsimd.memset(mr[:, :, 0], 0.0)
                nc.gpsimd.memset(mr[:, :, Wp - 1], 0.0)

                # ---- conv1 (5 passes/chunk) + bn1 + relu -> M1 ----
                for gi in range(ngroups):
                    g0 = gi * G
                    pss = [pspool.tile([128, N], F32, tag="ps",
                                       name=f"ps1_{img}_{g0}_{j}")
                           for j in range(G)]
                    for d in range(3):          # taps (d,0)+(d,1) from T2
                        for j in range(G):
                            h0 = (g0 + j) * R
                            nc.tensor.matmul(
                                pss[j][:, :],
                                lhsT=w1p2[:, d * 128:(d + 1) * 128],
                                rhs=t2[0:128, h0 + d:h0 + d + R, 0:W],
                                start=(d == 0), stop=False)
                    for j in range(G):          # taps (0,2)+(1,2) from T1
                        h0 = (g0 + j) * R
                        nc.tensor.matmul(
                            pss[j][:, :], lhsT=w1p1[:, :],
                            rhs=t1[0:128, h0:h0 + R, 2:2 + W],
                            start=False, stop=False)
                    for j in range(G):          # tap (2,2) from T1 upper
                        h0 = (g0 + j) * R
                        nc.tensor.matmul(
                            pss[j][:, :], lhsT=w1s[64:128, :],
                            rhs=t1[64:128, h0 + 1:h0 + 1 + R, 2:2 + W],
                            start=False, stop=True)
                    for j in range(G):
                        h0 = (g0 + j) * R
                        ps1 = pss[j]
                        p1lo = ps1[0:64, :].rearrange("p (h w) -> p h w", w=W)
                        p1hi = ps1[64:128, :].rearrange("p (h w) -> p h w", w=W)
                        # M half on ACT, M>>1 half on DVE (split so neither
                        # engine outruns the 5-pass PE group)
                        nc.scalar.activation(
                            mr[0:64, h0 + 1:h0 + 1 + R, 1:W + 1],
                            p1lo, AF.Relu, bias=b1t[0:64, 0:1])
                        nc.vector.tensor_scalar(
                            mr[64:128, h0:h0 + R, 1:W + 1],
                            p1hi, b1t[64:128, 0:1], 0.0, ALU.add, ALU.max)
                    # prefetch next image early: transfers overlap the rest
                    # of conv1 plus all of conv2 (split across two queues)
                    if img + 1 < B:
                        if gi == 1:
                            nxt = make_x_tiles(img + 1)
                            load_t2(nc.scalar, nxt[1], img + 1, [(0, Hp)])
                        elif gi == 3:
                            load_t1(nc.gpsimd, nxt[0], img + 1, [(0, Hp)])

                if img + 1 < B:
                    cur = nxt

                # ---- conv2 (6 passes/chunk) + bn2 + residual + relu ----
                for gi in range(ngroups):
                    g0 = gi * G
                    pss = [pspool.tile([64, N], F32, tag="ps",
                                       name=f"ps2_{img}_{g0}_{j}")
                           for j in range(G)]
                    for dw in range(3):
                        for j in range(G):
                            h0 = (g0 + j) * R
                            nc.tensor.matmul(
                                pss[j][:, :],
                                lhsT=w2p[:, dw * 64:(dw + 1) * 64],
                                rhs=mr[0:128, h0:h0 + R, dw:dw + W],
                                start=(dw == 0), stop=False)
                    for dw in range(3):
                        for j in range(G):
                            h0 = (g0 + j) * R
                            nc.tensor.matmul(
                                pss[j][:, :],
                                lhsT=w2s[64:128, dw * 64:(dw + 1) * 64],
                                rhs=mr[64:128, h0 + 1:h0 + 1 + R, dw:dw + W],
                                start=False, stop=(dw == 2))
                    o = opool.tile([64, G * N], F16, tag="o",
                                   name=f"o_{img}_{g0}")
                    for j in range(G):
                        h0 = (g0 + j) * R
                        u = upool.tile([64, N], F32, tag="u",
                                       name=f"u_{img}_{g0}_{j}")
                        nc.vector.tensor_add(
                            u[:, :].rearrange("p (h w) -> p h w", w=W),
                            pss[j][:, :].rearrange("p (h w) -> p h w", w=W),
                            t1[0:64, h0 + 1:h0 + 1 + R, 1:W + 1])
                        nc.vector.tensor_scalar(
                            o[:, j * N:(j + 1) * N], u[:, :],
                            b2t[:, 0:1], 0.0, ALU.add, ALU.max)
                    nc.sync.dma_start(
                        yout[img][:, g0 * N:(g0 + G) * N], o[:, :])
    nc.compile()
    return nc


def prepare_weights(w1, w2, alpha, bn1_gamma, bn1_beta, bn1_mean, bn1_var,
                    bn2_gamma, bn2_beta, bn2_mean, bn2_var):
    w1e = np.einsum('e,eoihw->oihw', alpha.astype(np.float64),
                    w1.astype(np.float64))
    w2e = np.einsum('e,eoihw->oihw', alpha.astype(np.float64),
                    w2.astype(np.float64))
    s1 = bn1_gamma / np.sqrt(bn1_var + EPS)
    b1 = bn1_beta - bn1_mean * s1
    s2 = bn2_gamma / np.sqrt(bn2_var + EPS)
    b2 = bn2_beta - bn2_mean * s2
    w1e = (w1e * s1[:, None, None, None]).astype(np.float16)  # fold bn1 scale
    w2e = (w2e * s2[:, None, None, None]).astype(np.float16)  # fold bn2 scale

    w1p2 = np.zeros((128, 3 * 128), np.float16)
    w1p1 = np.zeros((128, 128), np.float16)
    w1s = np.zeros((64, 128), np.float16)
    w2p = np.zeros((128, 3 * 64), np.float16)
    w2s = np.zeros((64, 3 * 64), np.float16)
    for d in range(3):
        for half in (0, 1):   # K half: T2 lower = tap (d,0), upper = (d,1)
            blk = w1e[:, :, d, half].T
            w1p2[half * 64:(half + 1) * 64, d * 128:d * 128 + 64] = blk
            w1p2[half * 64:(half + 1) * 64, d * 128 + 64:d * 128 + 128] = blk
    for half, dh in ((0, 0), (1, 1)):   # T1 lower = (0,2), upper = (1,2)
        blk = w1e[:, :, dh, 2].T
        w1p1[half * 64:(half + 1) * 64, 0:64] = blk
        w1p1[half * 64:(half + 1) * 64, 64:128] = blk
    w1s[:, 0:64] = w1e[:, :, 2, 2].T
    w1s[:, 64:128] = w1e[:, :, 2, 2].T
    for dw in range(3):
        for dh in (0, 1):
            w2p[dh * 64:(dh + 1) * 64, dw * 64:(dw + 1) * 64] = w2e[:, :, dh, dw].T
        w2s[:, dw * 64:(dw + 1) * 64] = w2e[:, :, 2, dw].T
    b1v = np.tile(b1.astype(np.float32), 2).reshape(128, 1)
    b2v = b2.astype(np.float32).reshape(64, 1)
    return {"w1p2": w1p2, "w1p1": w1p1, "w1s": w1s, "w2p": w2p, "w2s": w2s,
            "b1": b1v, "b2": b2v}


def prepare_x(x):
    """Zero-pad to [B, C, 114, 114] fp16, flattened per image."""
    B, C_, H, W = x.shape
    xp = np.zeros((B, C_, H + 2, W + 2), np.float16)
    xp[:, :, 1:H + 1, 1:W + 1] = x
    return np.ascontiguousarray(xp.reshape(B, C_, (H + 2) * (W + 2)))


def make_in_maps(x, wd):
    B_total = x.shape[0]
    Bc = B_total // N_CORES
    xp = prepare_x(np.asarray(x, np.float32))
    return [dict(wd, xin=np.ascontiguousarray(xp[cid * Bc:(cid + 1) * Bc]))
            for cid in range(N_CORES)]


_NC_CACHE = {}


def kernel(x, w1, w2, alpha,
           bn1_gamma, bn1_beta, bn1_mean, bn1_var,
           bn2_gamma, bn2_beta, bn2_mean, bn2_var):
    x = np.asarray(x, dtype=np.float32)
    B_total, _, H, W = x.shape
    Bc = B_total // N_CORES
    wd = prepare_weights(
        np.asarray(w1, np.float32), np.asarray(w2, np.float32),
        np.asarray(alpha, np.float32),
        np.asarray(bn1_gamma, np.float32), np.asarray(bn1_beta, np.float32),
        np.asarray(bn1_mean, np.float32), np.asarray(bn1_var, np.float32),
        np.asarray(bn2_gamma, np.float32), np.asarray(bn2_beta, np.float32),
        np.asarray(bn2_mean, np.float32), np.asarray(bn2_var, np.float32))

    key = (Bc, H, W)
    if key not in _NC_CACHE:
        _NC_CACHE[key] = build_nc(Bc, H, W)
    nc = _NC_CACHE[key]

    in_maps = make_in_maps(x, wd)
    res = run_bass_kernel_spmd(nc, in_maps, core_ids=list(range(N_CORES)))
    out = np.concatenate([res.results[cid]["yout"] for cid in range(N_CORES)],
                         axis=0)
    return out.reshape(B_total, 64, H, W).astype(np.float32)


# revision 7
# speedup vs baseline: 1.0271x; 1.0271x over previous
"""Trainium2 Bass kernel for nn_BasicBlock (MoE-combined residual conv block).

  out = relu(bn2(conv3x3(relu(bn1(conv3x3(x, w1e))), w2e)) + x)
  w{1,2}e = sum_e alpha[e] * w{1,2}[e]   (host-side: linear in weights)

Strategy (per NeuronCore, data-parallel over batch: 32 imgs -> 4 per core x 8):
  - x is shipped from the host already zero-padded to [64, 114, 114] and cast
    to fp16, flattened to [64, 12996]. Each image's conv input lives in SBUF
    as TWO 128-partition fp16 tiles loaded straight from HBM (no on-device
    casts, memsets or shuffles):
      T1 = [A ; A>>1row]   (upper = same HBM buffer at flat offset +114)
      T2 = [A ; A<<1col]   (upper = flat offset +1)
  - A matmul pass contracts 128 partitions = 2 planes x 64ch; with row
    offsets of T1/T2 a 3x3 conv needs only FIVE passes per output chunk
    (PE cost is N_free cycles regardless of K, so pass count is everything):
      3x T2@rowoff d: taps (d,0)+(d,1)   [K=128]
      1x T1@rowoff 0, coloff 2: taps (0,2)+(1,2)   [K=128]
      1x T1 upper@rowoff 1, coloff 2: tap (2,2)    [K=64]
  - conv2 keeps the 6-pass structure on M1 = [M ; M>>1row] (3 pair passes +
    3 singles) so the mid plane needs no extra DMA copies: ACT evicts the M
    half (relu+bn1 fused, scale folded into w1), DVE evicts the M>>1 half --
    split across engines so neither outruns the PE group time.
  - conv2 epilogue on DVE: residual add (fp16 x from T1 lower) straight out
    of PSUM, then relu (+bn2 bias; bn2 scale folded into w2) into a per-group
    fp16 tile, one DMA per 16-row group to HBM. Host upcasts to fp32.
"""

import numpy as np

import concourse.mybir as mybir
import concourse.tile as tile
from concourse import bacc
from concourse.bass_utils import run_bass_kernel_spmd

F32 = mybir.dt.float32
F16 = mybir.dt.float16
AF = mybir.ActivationFunctionType
ALU = mybir.AluOpType

EPS = 1e-5
N_CORES = 8
C = 64   # channels (in == out)
R = 4    # output rows per PSUM chunk
G = 4    # chunks per weight-stationary group


def build_nc(B, H, W):
    """Bass program: B images of [64, H, W] per core, pre-padded fp16 input."""
    Hp, Wp = H + 2, W + 2
    FLAT = Hp * Wp
    N = R * W                     # psum free size per chunk
    nchunks = H // R
    assert H % R == 0 and nchunks % G == 0
    ngroups = nchunks // G

    nc = bacc.Bacc("TRN2", target_bir_lowering=False, debug=False,
                   enable_asserts=False, num_devices=N_CORES)

    xin = nc.dram_tensor("xin", [B, C, FLAT], F16, kind="ExternalInput").ap()
    w1p2_d = nc.dram_tensor("w1p2", [128, 3 * 128], F16, kind="ExternalInput").ap()
    w1p1_d = nc.dram_tensor("w1p1", [128, 128], F16, kind="ExternalInput").ap()
    w1s_d = nc.dram_tensor("w1s", [64, 128], F16, kind="ExternalInput").ap()
    w2p_d = nc.dram_tensor("w2p", [128, 3 * 64], F16, kind="ExternalInput").ap()
    w2s_d = nc.dram_tensor("w2s", [64, 3 * 64], F16, kind="ExternalInput").ap()
    b1_d = nc.dram_tensor("b1", [128, 1], F32, kind="ExternalInput").ap()
    b2_d = nc.dram_tensor("b2", [64, 1], F32, kind="ExternalInput").ap()
    yout = nc.dram_tensor("yout", [B, C, H * W], F16, kind="ExternalOutput").ap()

    with tile.TileContext(nc) as tc:
        with (
            tc.tile_pool(name="wpool", bufs=1) as wpool,
            tc.tile_pool(name="xpool", bufs=2) as xpool,
            tc.tile_pool(name="mpool", bufs=1) as mpool,
            tc.tile_pool(name="pspool", bufs=8, space="PSUM") as pspool,
            tc.tile_pool(name="upool", bufs=4) as upool,
            tc.tile_pool(name="opool", bufs=3) as opool,
        ):
            w1p2 = wpool.tile([128, 3 * 128], F16)
            w1p1 = wpool.tile([128, 128], F16)
            w1s = wpool.tile([128, 128], F16)   # content in parts 64-127
            w2p = wpool.tile([128, 3 * 64], F16)
            w2s = wpool.tile([128, 3 * 64], F16)  # content in parts 64-127
            b1t = wpool.tile([128, 1], F32)
            b2t = wpool.tile([64, 1], F32)
            # w1p2 + b1t first (needed by group 0); conv2 weights last
            nc.sync.dma_start(w1p2[:, :], w1p2_d[:, :])
            nc.sync.dma_start(b1t[:, :], b1_d[:, :])
            nc.sync.dma_start(w1p1[:, :], w1p1_d[:, :])
            nc.sync.dma_start(w1s[64:128, :], w1s_d[:, :])
            nc.sync.dma_start(w2p[:, :], w2p_d[:, :])
            nc.sync.dma_start(w2s[64:128, :], w2s_d[:, :])
            nc.sync.dma_start(b2t[:, :], b2_d[:, :])

            def make_x_tiles(img):
                t1 = xpool.tile([128, FLAT], F16, tag="t1", name=f"t1_{img}")
                t2 = xpool.tile([128, FLAT], F16, tag="t2", name=f"t2_{img}")
                return t1, t2

            def load_t2(eng, t2, img, bands):
                src = xin[img]
                for r0, r1 in bands:
                    a, b = r0 * Wp, r1 * Wp
                    eng.dma_start(t2[0:64, a:b], src[:, a:b])
                    bb = min(b, FLAT - 1)
                    eng.dma_start(t2[64:128, a:bb], src[:, a + 1:bb + 1])

            def load_t1(eng, t1, img, bands):
                src = xin[img]
                for r0, r1 in bands:
                    a, b = r0 * Wp, r1 * Wp
                    eng.dma_start(t1[0:64, a:b], src[:, a:b])
                    bb = min(b, FLAT - Wp)
                    if bb > a:
                        eng.dma_start(t1[64:128, a:bb], src[:, a + Wp:bb + Wp])

            # image 0: tiny first band leads each queue so its transfer isn't
            # stuck behind the bulk bands in the shared DMA engine pool; the
            # rest streams progressively, T2 and T1 on separate queues.
            cur = make_x_tiles(0)
            bands0 = [(0, 6), (6, 16), (16, 32), (32, 56), (56, 86), (86, Hp)]
            load_t2(nc.scalar, cur[1], 0, bands0)
            load_t1(nc.gpsimd, cur[0], 0, bands0)

            for img in range(B):
                t1f, t2f = cur
                t1 = t1f[:, :].rearrange("p (h w) -> p h w", w=Wp)
                t2 = t2f[:, :].rearrange("p (h w) -> p h w", w=Wp)

                # ---- mid plane M1 = [M ; M>>1], written by ACT/DVE ----
                mt = mpool.tile([128, FLAT], F16, tag="mt", name=f"mt_{img}")
                mr = mt[:, :].rearrange("p (h w) -> p h w", w=Wp)
                nc.gpsimd.memset(mr[0:64, 0, :], 0.0)
                nc.gpsimd.memset(mr[0:64, Hp - 1, :], 0.0)
                nc.gpsimd.memset(mr[64:128, H, :], 0.0)
                nc.gpsimd.memset(mr[:, :, 0], 0.0)
                nc.gpsimd.memset(mr[:, :, Wp - 1], 0.0)

                # ---- conv1 (5 passes/chunk) + bn1 + relu -> M1 ----
                for gi in range(ngroups):
                    g0 = gi * G
                    pss = [pspool.tile([128, N], F32, tag="ps",
                                       name=f"ps1_{img}_{g0}_{j}")
                           for j in range(G)]
                    for d in range(3):          # taps (d,0)+(d,1) from T2
                        for j in range(G):
                            h0 = (g0 + j) * R
                            nc.tensor.matmul(
                                pss[j][:, :],
                                lhsT=w1p2[:, d * 128:(d + 1) * 128],
                                rhs=t2[0:128, h0 + d:h0 + d + R, 0:W],
                                start=(d == 0), stop=False)
                    for j in range(G):          # taps (0,2)+(1,2) from T1
                        h0 = (g0 + j) * R
                        nc.tensor.matmul(
                            pss[j][:, :], lhsT=w1p1[:, :],
                            rhs=t1[0:128, h0:h0 + R, 2:2 + W],
                            start=False, stop=False)
                    for j in range(G):          # tap (2,2) from T1 upper
                        h0 = (g0 + j) * R
                        nc.tensor.matmul(
                            pss[j][:, :], lhsT=w1s[64:128, :],
                            rhs=t1[64:128, h0 + 1:h0 + 1 + R, 2:2 + W],
                            start=False, stop=True)
                    for j in range(G):
                        h0 = (g0 + j) * R
                        ps1 = pss[j]
                        p1lo = ps1[0:64, :].rearrange("p (h w) -> p h w", w=W)
                        p1hi = ps1[64:128, :].rearrange("p (h w) -> p h w", w=W)
                        # M half on ACT, M>>1 half on DVE (split so neither
                        # engine outruns the 5-pass PE group)
                        nc.scalar.activation(
                            mr[0:64, h0 + 1:h0 + 1 + R, 1:W + 1],
                            p1lo, AF.Relu, bias=b1t[0:64, 0:1])
                        nc.vector.tensor_scalar(
                            mr[64:128, h0:h0 + R, 1:W + 1],
                            p1hi, b1t[64:128, 0:1], 0.0, ALU.add, ALU.max)
                    # prefetch next image early: transfers overlap the rest
                    # of conv1 plus all of conv2 (split across two queues)
                    if img + 1 < B:
                        if gi == 1:
                            nxt = make_x_tiles(img + 1)
                            load_t2(nc.scalar, nxt[1], img + 1, [(0, Hp)])
                        elif gi == 3:
                            load_t1(nc.gpsimd, nxt[0], img + 1, [(0, Hp)])

                if img + 1 < B:
                    cur = nxt

                # ---- conv2 (6 passes/chunk) + bn2 + residual + relu ----
                for gi in range(ngroups):
                    g0 = gi * G
                    pss = [pspool.tile([64, N], F32, tag="ps",
                                       name=f"ps2_{img}_{g0}_{j}")
                           for j in range(G)]
                    for dw in range(3):
                        for j in range(G):
                            h0 = (g0 + j) * R
                            nc.tensor.matmul(
                                pss[j][:, :],
                                lhsT=w2p[:, dw * 64:(dw + 1) * 64],
                                rhs=mr[0:128, h0:h0 + R, dw:dw + W],
                                start=(dw == 0), stop=False)
                    for dw in range(3):
                        for j in range(G):
                            h0 = (g0 + j) * R
                            nc.tensor.matmul(
                                pss[j][:, :],
                                lhsT=w2s[64:128, dw * 64:(dw + 1) * 64],
                                rhs=mr[64:128, h0 + 1:h0 + 1 + R, dw:dw + W],
                                start=False, stop=(dw == 2))
                    o = opool.tile([64, G * N], F16, tag="o",
                                   name=f"o_{img}_{g0}")
                    for j in range(G):
                        h0 = (g0 + j) * R
                        u = upool.tile([64, N], F32, tag="u",
                                       name=f"u_{img}_{g0}_{j}")
                        nc.vector.tensor_add(
                            u[:, :].rearrange("p (h w) -> p h w", w=W),
                            pss[j][:, :].rearrange("p (h w) -> p h w", w=W),
                            t1[0:64, h0 + 1:h0 + 1 + R, 1:W + 1])
                        nc.vector.tensor_scalar(
                            o[:, j * N:(j + 1) * N], u[:, :],
                            b2t[:, 0:1], 0.0, ALU.add, ALU.max)
                    nc.sync.dma_start(
                        yout[img][:, g0 * N:(g0 + G) * N], o[:, :])
    nc.compile()
    return nc


def prepare_weights(w1, w2, alpha, bn1_gamma, bn1_beta, bn1_mean, bn1_var,
                    bn2_gamma, bn2_beta, bn2_mean, bn2_var):
    w1e = np.einsum('e,eoihw->oihw', alpha.astype(np.float64),
                    w1.astype(np.float64))
    w2e = np.einsum('e,eoihw->oihw', alpha.astype(np.float64),
                    w2.astype(np.float64))
    s1 = bn1_gamma / np.sqrt(bn1_var + EPS)
    b1 = bn1_beta - bn1_mean * s1
    s2 = bn2_gamma / np.sqrt(bn2_var + EPS)
    b2 = bn2_beta - bn2_mean * s2
    w1e = (w1e * s1[:, None, None, None]).astype(np.float16)  # fold bn1 scale
    w2e = (w2e * s2[:, None, None, None]).astype(np.float16)  # fold bn2 scale

    w1p2 = np.zeros((128, 3 * 128), np.float16)
    w1p1 = np.zeros((128, 128), np.float16)
    w1s = np.zeros((64, 128), np.float16)
    w2p = np.zeros((128, 3 * 64), np.float16)
    w2s = np.zeros((64, 3 * 64), np.float16)
    for d in range(3):
        for half in (0, 1):   # K half: T2 lower = tap (d,0), upper = (d,1)
            blk = w1e[:, :, d, half].T
            w1p2[half * 64:(half + 1) * 64, d * 128:d * 128 + 64] = blk
            w1p2[half * 64:(half + 1) * 64, d * 128 + 64:d * 128 + 128] = blk
    for half, dh in ((0, 0), (1, 1)):   # T1 lower = (0,2), upper = (1,2)
        blk = w1e[:, :, dh, 2].T
        w1p1[half * 64:(half + 1) * 64, 0:64] = blk
        w1p1[half * 64:(half + 1) * 64, 64:128] = blk
    w1s[:, 0:64] = w1e[:, :, 2, 2].T
    w1s[:, 64:128] = w1e[:, :, 2, 2].T
    for dw in range(3):
        for dh in (0, 1):
            w2p[dh * 64:(dh + 1) * 64, dw * 64:(dw + 1) * 64] = w2e[:, :, dh, dw].T
        w2s[:, dw * 64:(dw + 1) * 64] = w2e[:, :, 2, dw].T
    b1v = np.tile(b1.astype(np.float32), 2).reshape(128, 1)
    b2v = b2.astype(np.float32).reshape(64, 1)
    return {"w1p2": w1p2, "w1p1": w1p1, "w1s": w1s, "w2p": w2p, "w2s": w2s,
            "b1": b1v, "b2": b2v}


def prepare_x(x):
    """Zero-pad to [B, C, 114, 114] fp16, flattened per image."""
    B, C_, H, W = x.shape
    xp = np.zeros((B, C_, H + 2, W + 2), np.float16)
    xp[:, :, 1:H + 1, 1:W + 1] = x
    return np.ascontiguousarray(xp.reshape(B, C_, (H + 2) * (W + 2)))


def make_in_maps(x, wd):
    B_total = x.shape[0]
    Bc = B_total // N_CORES
    xp = prepare_x(np.asarray(x, np.float32))
    return [dict(wd, xin=np.ascontiguousarray(xp[cid * Bc:(cid + 1) * Bc]))
            for cid in range(N_CORES)]


_NC_CACHE = {}


def kernel(x, w1, w2, alpha,
           bn1_gamma, bn1_beta, bn1_mean, bn1_var,
           bn2_gamma, bn2_beta, bn2_mean, bn2_var):
    x = np.asarray(x, dtype=np.float32)
    B_total, _, H, W = x.shape
    Bc = B_total // N_CORES
    wd = prepare_weights(
        np.asarray(w1, np.float32), np.asarray(w2, np.float32),
        np.asarray(alpha, np.float32),
        np.asarray(bn1_gamma, np.float32), np.asarray(bn1_beta, np.float32),
        np.asarray(bn1_mean, np.float32), np.asarray(bn1_var, np.float32),
        np.asarray(bn2_gamma, np.float32), np.asarray(bn2_beta, np.float32),
        np.asarray(bn2_mean, np.float32), np.asarray(bn2_var, np.float32))

    key = (Bc, H, W)
    if key not in _NC_CACHE:
        _NC_CACHE[key] = build_nc(Bc, H, W)
    nc = _NC_CACHE[key]

    in_maps = make_in_maps(x, wd)
    res = run_bass_kernel_spmd(nc, in_maps, core_ids=list(range(N_CORES)))
    out = np.concatenate([res.results[cid]["yout"] for cid in range(N_CORES)],
                         axis=0)
    return out.reshape(B_total, 64, H, W).astype(np.float32)


# revision 10
# speedup vs baseline: 1.0780x; 1.0496x over previous
"""Trainium2 Bass kernel for nn_BasicBlock (MoE-combined residual conv block).

  out = relu(bn2(conv3x3(relu(bn1(conv3x3(x, w1e))), w2e)) + x)
  w{1,2}e = sum_e alpha[e] * w{1,2}[e]   (host-side: linear in weights)

Strategy (per NeuronCore, data-parallel over batch: 32 imgs -> 4 per core x 8):
  - x is shipped from the host already zero-padded to [64, 114, 114] and cast
    to fp16, flattened to [64, 12996]. Each image's conv input lives in SBUF
    as TWO 128-partition fp16 tiles loaded straight from HBM (no on-device
    casts, memsets or shuffles):
      T1 = [A ; A>>1row]   (upper = same HBM buffer at flat offset +114)
      T2 = [A ; A<<1col]   (upper = flat offset +1)
  - A matmul pass contracts 128 partitions = 2 planes x 64ch; with row
    offsets of T1/T2 a 3x3 conv needs only FIVE passes per output chunk
    (PE cost is N_free cycles regardless of K, so pass count is everything):
      3x T2@rowoff d: taps (d,0)+(d,1)   [K=128]
      1x T1@rowoff 0, coloff 2: taps (0,2)+(1,2)   [K=128]
      1x T1 upper@rowoff 1, coloff 2: tap (2,2)    [K=64]
  - conv2 keeps the 6-pass structure on M1 = [M ; M>>1row] (3 pair passes +
    3 singles) so the mid plane needs no extra DMA copies: ACT evicts the M
    half (relu+bn1 fused, scale folded into w1), DVE evicts the M>>1 half --
    split across engines so neither outruns the PE group time.
  - conv2 epilogue on DVE: residual add (fp16 x from T1 lower) straight out
    of PSUM, then relu (+bn2 bias; bn2 scale folded into w2) into a per-group
    fp16 tile, one DMA per 16-row group to HBM. Host upcasts to fp32.
"""

import numpy as np

import concourse.mybir as mybir
import concourse.tile as tile
from concourse import bacc
from concourse.bass_utils import run_bass_kernel_spmd

F32 = mybir.dt.float32
F16 = mybir.dt.float16
AF = mybir.ActivationFunctionType
ALU = mybir.AluOpType

EPS = 1e-5
N_CORES = 8
C = 64   # channels (in == out)
R = 4    # output rows per PSUM chunk
G = 4    # chunks per weight-stationary group


def build_nc(B, H, W):
    """Bass program: B images of [64, H, W] per core, pre-padded fp16 input."""
    Hp, Wp = H + 2, W + 2
    FLAT = Hp * Wp
    N = R * W                     # psum free size per chunk
    nchunks = H // R
    assert H % R == 0 and nchunks % G == 0
    ngroups = nchunks // G

    nc = bacc.Bacc("TRN2", target_bir_lowering=False, debug=False,
                   enable_asserts=False, num_devices=N_CORES)

    xin = nc.dram_tensor("xin", [B, C, FLAT], F16, kind="ExternalInput").ap()
    w1p2_d = nc.dram_tensor("w1p2", [128, 3 * 128], F16, kind="ExternalInput").ap()
    w1p1_d = nc.dram_tensor("w1p1", [128, 128], F16, kind="ExternalInput").ap()
    w1s_d = nc.dram_tensor("w1s", [64, 128], F16, kind="ExternalInput").ap()
    w2p_d = nc.dram_tensor("w2p", [128, 3 * 64], F16, kind="ExternalInput").ap()
    w2s_d = nc.dram_tensor("w2s", [64, 3 * 64], F16, kind="ExternalInput").ap()
    b1_d = nc.dram_tensor("b1", [128, 1], F32, kind="ExternalInput").ap()
    b2_d = nc.dram_tensor("b2", [64, 1], F32, kind="ExternalInput").ap()
    yout = nc.dram_tensor("yout", [B, C, H * W], F16, kind="ExternalOutput").ap()

    with tile.TileContext(nc) as tc:
        with (
            tc.tile_pool(name="wpool", bufs=1) as wpool,
            tc.tile_pool(name="xpool", bufs=2) as xpool,
            tc.tile_pool(name="mpool", bufs=1) as mpool,
            tc.tile_pool(name="pspool", bufs=8, space="PSUM") as pspool,
            tc.tile_pool(name="upool", bufs=4) as upool,
            tc.tile_pool(name="opool", bufs=3) as opool,
        ):
            w1p2 = wpool.tile([128, 3 * 128], F16)
            w1p1 = wpool.tile([128, 128], F16)
            w1s = wpool.tile([128, 128], F16)   # content in parts 64-127
            w2p = wpool.tile([128, 3 * 64], F16)
            w2s = wpool.tile([128, 3 * 64], F16)  # content in parts 64-127
            b1t = wpool.tile([128, 1], F32)
            b2t = wpool.tile([64, 1], F32)
            # w1p2 + b1t first (needed by group 0); conv2 weights last
            nc.sync.dma_start(w1p2[:, :], w1p2_d[:, :])
            nc.sync.dma_start(b1t[:, :], b1_d[:, :])
            nc.sync.dma_start(w1p1[:, :], w1p1_d[:, :])
            nc.sync.dma_start(w1s[64:128, :], w1s_d[:, :])
            nc.sync.dma_start(w2p[:, :], w2p_d[:, :])
            nc.sync.dma_start(w2s[64:128, :], w2s_d[:, :])
            nc.sync.dma_start(b2t[:, :], b2_d[:, :])

            def make_x_tiles(img):
                t1 = xpool.tile([128, FLAT], F16, tag="t1", name=f"t1_{img}")
                t2 = xpool.tile([128, FLAT], F16, tag="t2", name=f"t2_{img}")
                return t1, t2

            def load_t2(eng, t2, img, bands):
                src = xin[img]
                for r0, r1 in bands:
                    a, b = r0 * Wp, r1 * Wp
                    eng.dma_start(t2[0:64, a:b], src[:, a:b])
                    bb = min(b, FLAT - 1)
                    eng.dma_start(t2[64:128, a:bb], src[:, a + 1:bb + 1])

            def load_t1(eng, t1, img, bands):
                src = xin[img]
                for r0, r1 in bands:
                    a, b = r0 * Wp, r1 * Wp
                    eng.dma_start(t1[0:64, a:b], src[:, a:b])
                    bb = min(b, FLAT - Wp)
                    if bb > a:
                        eng.dma_start(t1[64:128, a:bb], src[:, a + Wp:bb + Wp])

            # image 0: tiny first band leads each queue so its transfer isn't
            # stuck behind the bulk bands in the shared DMA engine pool; the
            # rest streams progressively, T2 and T1 on separate queues.
            cur = make_x_tiles(0)
            bands0 = [(0, 6), (6, 16), (16, 32), (32, 56), (56, 86), (86, Hp)]
            load_t2(nc.scalar, cur[1], 0, bands0)
            load_t1(nc.gpsimd, cur[0], 0, bands0)

            for img in range(B):
                t1f, t2f = cur
                t1 = t1f[:, :].rearrange("p (h w) -> p h w", w=Wp)
                t2 = t2f[:, :].rearrange("p (h w) -> p h w", w=Wp)

                # ---- mid plane M1 = [M ; M>>1], written by ACT/DVE ----
                mt = mpool.tile([128, FLAT], F16, tag="mt", name=f"mt_{img}")
                mr = mt[:, :].rearrange("p (h w) -> p h w", w=Wp)
                nc.gpsimd.memset(mr[0:64, 0, :], 0.0)
                nc.gpsimd.memset(mr[0:64, Hp - 1, :], 0.0)
                nc.gpsimd.memset(mr[64:128, H, :], 0.0)
                nc.gpsimd.memset(mr[:, :, 0], 0.0)
                nc.gpsimd.memset(mr[:, :, Wp - 1], 0.0)

                # ---- conv1 (5 passes/chunk) + bn1 + relu -> M1 ----
                # pass k: 0-2 = taps (k,0)+(k,1) from T2; 3 = (0,2)+(1,2)
                # from T1; 4 = (2,2) from T1 upper. Palindrome order across
                # groups so consecutive groups share the boundary stationary.
                def c1_pass(k, j, g0, pss, start, stop):
                    h0 = (g0 + j) * R
                    if k < 3:
                        nc.tensor.matmul(
                            pss[j][:, :],
                            lhsT=w1p2[:, k * 128:(k + 1) * 128],
                            rhs=t2[0:128, h0 + k:h0 + k + R, 0:W],
                            start=start, stop=stop)
                    elif k == 3:
                        nc.tensor.matmul(
                            pss[j][:, :], lhsT=w1p1[:, :],
                            rhs=t1[0:128, h0:h0 + R, 2:2 + W],
                            start=start, stop=stop)
                    else:
                        nc.tensor.matmul(
                            pss[j][:, :], lhsT=w1s[64:128, :],
                            rhs=t1[64:128, h0 + 1:h0 + 1 + R, 2:2 + W],
                            start=start, stop=stop)

                for gi in range(ngroups):
                    g0 = gi * G
                    pss = [pspool.tile([128, N], F32, tag="ps",
                                       name=f"ps1_{img}_{g0}_{j}")
                           for j in range(G)]
                    order = range(5) if gi % 2 == 0 else range(4, -1, -1)
                    for ki, k in enumerate(order):
                        for j in range(G):
                            c1_pass(k, j, g0, pss, ki == 0, ki == 4)
                    for j in range(G):
                        h0 = (g0 + j) * R
                        ps1 = pss[j]
                        p1lo = ps1[0:64, :].rearrange("p (h w) -> p h w", w=W)
                        p1hi = ps1[64:128, :].rearrange("p (h w) -> p h w", w=W)
                        # M half on ACT, M>>1 half on DVE (split so neither
                        # engine outruns the 5-pass PE group)
                        nc.scalar.activation(
                            mr[0:64, h0 + 1:h0 + 1 + R, 1:W + 1],
                            p1lo, AF.Relu, bias=b1t[0:64, 0:1])
                        nc.vector.tensor_scalar(
                            mr[64:128, h0:h0 + R, 1:W + 1],
                            p1hi, b1t[64:128, 0:1], 0.0, ALU.add, ALU.max)
                    # prefetch next image in bands so its early rows aren't
                    # stuck behind this image's full planes in the DMA pool
                    if img + 1 < B:
                        if gi == 1:
                            nxt = make_x_tiles(img + 1)
                            load_t2(nc.scalar, nxt[1], img + 1, [(0, 38)])
                        elif gi == 3:
                            load_t2(nc.scalar, nxt[1], img + 1, [(38, 76)])
                            load_t1(nc.gpsimd, nxt[0], img + 1, [(0, 38)])
                        elif gi == 5:
                            load_t2(nc.scalar, nxt[1], img + 1, [(76, Hp)])
                            load_t1(nc.gpsimd, nxt[0], img + 1, [(38, 76)])

                if img + 1 < B:
                    load_t1(nc.gpsimd, nxt[0], img + 1, [(76, Hp)])
                    cur = nxt

                # ---- conv2 (6 passes/chunk) + bn2 + residual + relu ----
                def c2_pass(k, j, g0, pss, start, stop):
                    h0 = (g0 + j) * R
                    if k < 3:
                        nc.tensor.matmul(
                            pss[j][:, :],
                            lhsT=w2p[:, k * 64:(k + 1) * 64],
                            rhs=mr[0:128, h0:h0 + R, k:k + W],
                            start=start, stop=stop)
                    else:
                        dw = k - 3
                        nc.tensor.matmul(
                            pss[j][:, :],
                            lhsT=w2s[64:128, dw * 64:(dw + 1) * 64],
                            rhs=mr[64:128, h0 + 1:h0 + 1 + R, dw:dw + W],
                            start=start, stop=stop)

                for gi in range(ngroups):
                    g0 = gi * G
                    pss = [pspool.tile([64, N], F32, tag="ps",
                                       name=f"ps2_{img}_{g0}_{j}")
                           for j in range(G)]
                    order = range(6) if gi % 2 == 0 else range(5, -1, -1)
                    for ki, k in enumerate(order):
                        for j in range(G):
                            c2_pass(k, j, g0, pss, ki == 0, ki == 5)
                    o = opool.tile([64, G * N], F16, tag="o",
                                   name=f"o_{img}_{g0}")
                    for j in range(G):
                        h0 = (g0 + j) * R
                        u = upool.tile([64, N], F32, tag="u",
                                       name=f"u_{img}_{g0}_{j}")
                        nc.vector.tensor_add(
                            u[:, :].rearrange("p (h w) -> p h w", w=W),
                            pss[j][:, :].rearrange("p (h w) -> p h w", w=W),
                            t1[0:64, h0 + 1:h0 + 1 + R, 1:W + 1])
                        nc.vector.tensor_scalar(
                            o[:, j * N:(j + 1) * N], u[:, :],
                            b2t[:, 0:1], 0.0, ALU.add, ALU.max)
                    nc.sync.dma_start(
                        yout[img][:, g0 * N:(g0 + G) * N], o[:, :])
    nc.compile()
    return nc


def prepare_weights(w1, w2, alpha, bn1_gamma, bn1_beta, bn1_mean, bn1_var,
                    bn2_gamma, bn2_beta, bn2_mean, bn2_var):
    w1e = np.einsum('e,eoihw->oihw', alpha.astype(np.float64),
                    w1.astype(np.float64))
    w2e = np.einsum('e,eoihw->oihw', alpha.astype(np.float64),
                    w2.astype(np.float64))
    s1 = bn1_gamma / np.sqrt(bn1_var + EPS)
    b1 = bn1_beta - bn1_mean * s1
    s2 = bn2_gamma / np.sqrt(bn2_var + EPS)
    b2 = bn2_beta - bn2_mean * s2
    w1e = (w1e * s1[:, None, None, None]).astype(np.float16)  # fold bn1 scale
    w2e = (w2e * s2[:, None, None, None]).astype(np.float16)  # fold bn2 scale

    w1p2 = np.zeros((128, 3 * 128), np.float16)
    w1p1 = np.zeros((128, 128), np.float16)
    w1s = np.zeros((64, 128), np.float16)
    w2p = np.zeros((128, 3 * 64), np.float16)
    w2s = np.zeros((64, 3 * 64), np.float16)
    for d in range(3):
        for half in (0, 1):   # K half: T2 lower = tap (d,0), upper = (d,1)
            blk = w1e[:, :, d, half].T
            w1p2[half * 64:(half + 1) * 64, d * 128:d * 128 + 64] = blk
            w1p2[half * 64:(half + 1) * 64, d * 128 + 64:d * 128 + 128] = blk
    for half, dh in ((0, 0), (1, 1)):   # T1 lower = (0,2), upper = (1,2)
        blk = w1e[:, :, dh, 2].T
        w1p1[half * 64:(half + 1) * 64, 0:64] = blk
        w1p1[half * 64:(half + 1) * 64, 64:128] = blk
    w1s[:, 0:64] = w1e[:, :, 2, 2].T
    w1s[:, 64:128] = w1e[:, :, 2, 2].T
    for dw in range(3):
        for dh in (0, 1):
            w2p[dh * 64:(dh + 1) * 64, dw * 64:(dw + 1) * 64] = w2e[:, :, dh, dw].T
        w2s[:, dw * 64:(dw + 1) * 64] = w2e[:, :, 2, dw].T
    b1v = np.tile(b1.astype(np.float32), 2).reshape(128, 1)
    b2v = b2.astype(np.float32).reshape(64, 1)
    return {"w1p2": w1p2, "w1p1": w1p1, "w1s": w1s, "w2p": w2p, "w2s": w2s,
            "b1": b1v, "b2": b2v}


def prepare_x(x):
    """Zero-pad to [B, C, 114, 114] fp16, flattened per image."""
    B, C_, H, W = x.shape
    xp = np.zeros((B, C_, H + 2, W + 2), np.float16)
    xp[:, :, 1:H + 1, 1:W + 1] = x
    return np.ascontiguousarray(xp.reshape(B, C_, (H + 2) * (W + 2)))


def make_in_maps(x, wd):
    B_total = x.shape[0]
    Bc = B_total // N_CORES
    xp = prepare_x(np.asarray(x, np.float32))
    return [dict(wd, xin=np.ascontiguousarray(xp[cid * Bc:(cid + 1) * Bc]))
            for cid in range(N_CORES)]


_NC_CACHE = {}


def kernel(x, w1, w2, alpha,
           bn1_gamma, bn1_beta, bn1_mean, bn1_var,
           bn2_gamma, bn2_beta, bn2_mean, bn2_var):
    x = np.asarray(x, dtype=np.float32)
    B_total, _, H, W = x.shape
    Bc = B_total // N_CORES
    wd = prepare_weights(
        np.asarray(w1, np.float32), np.asarray(w2, np.float32),
        np.asarray(alpha, np.float32),
        np.asarray(bn1_gamma, np.float32), np.asarray(bn1_beta, np.float32),
        np.asarray(bn1_mean, np.float32), np.asarray(bn1_var, np.float32),
        np.asarray(bn2_gamma, np.float32), np.asarray(bn2_beta, np.float32),
        np.asarray(bn2_mean, np.float32), np.asarray(bn2_var, np.float32))

    key = (Bc, H, W)
    if key not in _NC_CACHE:
        _NC_CACHE[key] = build_nc(Bc, H, W)
    nc = _NC_CACHE[key]

    in_maps = make_in_maps(x, wd)
    res = run_bass_kernel_spmd(nc, in_maps, core_ids=list(range(N_CORES)))
    out = np.concatenate([res.results[cid]["yout"] for cid in range(N_CORES)],
                         axis=0)
    return out.reshape(B_total, 64, H, W).astype(np.float32)


# revision 11
# speedup vs baseline: 1.1047x; 1.0248x over previous
"""Trainium2 Bass kernel for nn_BasicBlock (MoE-combined residual conv block).

  out = relu(bn2(conv3x3(relu(bn1(conv3x3(x, w1e))), w2e)) + x)
  w{1,2}e = sum_e alpha[e] * w{1,2}[e]   (host-side: linear in weights)

Strategy (per NeuronCore, data-parallel over batch: 32 imgs -> 4 per core x 8):
  - x is shipped from the host already zero-padded to [64, 114, 114] and cast
    to fp16, flattened to [64, 12996]. Each image's conv input lives in SBUF
    as TWO 128-partition fp16 tiles loaded straight from HBM (no on-device
    casts, memsets or shuffles):
      T1 = [A ; A>>1row]   (upper = same HBM buffer at flat offset +114)
      T2 = [A ; A<<1col]   (upper = flat offset +1)
  - A matmul pass contracts 128 partitions = 2 planes x 64ch; with row
    offsets of T1/T2 a 3x3 conv needs only FIVE passes per output chunk
    (PE cost is N_free cycles regardless of K, so pass count is everything):
      3x T2@rowoff d: taps (d,0)+(d,1)   [K=128]
      1x T1@rowoff 0, coloff 2: taps (0,2)+(1,2)   [K=128]
      1x T1 upper@rowoff 1, coloff 2: tap (2,2)    [K=64]
  - conv2 keeps the 6-pass structure on M1 = [M ; M>>1row] (3 pair passes +
    3 singles) so the mid plane needs no extra DMA copies: ACT evicts the M
    half (relu+bn1 fused, scale folded into w1), DVE evicts the M>>1 half --
    split across engines so neither outruns the PE group time.
  - conv2 epilogue on DVE: residual add (fp16 x from T1 lower) straight out
    of PSUM, then relu (+bn2 bias; bn2 scale folded into w2) into a per-group
    fp16 tile, one DMA per 16-row group to HBM. Host upcasts to fp32.
"""

import numpy as np

import concourse.mybir as mybir
import concourse.tile as tile
from concourse import bacc
from concourse.bass_utils import run_bass_kernel_spmd

F32 = mybir.dt.float32
F16 = mybir.dt.float16
AF = mybir.ActivationFunctionType
ALU = mybir.AluOpType

EPS = 1e-5
N_CORES = 8
C = 64   # channels (in == out)
R = 4    # output rows per PSUM chunk
G = 4    # chunks per weight-stationary group


def build_nc(B, H, W):
    """Bass program: B images of [64, H, W] per core, pre-padded fp16 input."""
    Hp, Wp = H + 2, W + 2
    FLAT = Hp * Wp
    N = R * W                     # psum free size per chunk
    nchunks = H // R
    assert H % R == 0 and nchunks % G == 0
    ngroups = nchunks // G

    nc = bacc.Bacc("TRN2", target_bir_lowering=False, debug=False,
                   enable_asserts=False, num_devices=N_CORES)

    xin = nc.dram_tensor("xin", [B, C, FLAT], F16, kind="ExternalInput").ap()
    w1p2_d = nc.dram_tensor("w1p2", [128, 3 * 128], F16, kind="ExternalInput").ap()
    w1p1_d = nc.dram_tensor("w1p1", [128, 128], F16, kind="ExternalInput").ap()
    w1s_d = nc.dram_tensor("w1s", [64, 128], F16, kind="ExternalInput").ap()
    w2p_d = nc.dram_tensor("w2p", [128, 3 * 64], F16, kind="ExternalInput").ap()
    w2s_d = nc.dram_tensor("w2s", [64, 3 * 64], F16, kind="ExternalInput").ap()
    b1_d = nc.dram_tensor("b1", [128, 1], F32, kind="ExternalInput").ap()
    b2_d = nc.dram_tensor("b2", [64, 1], F32, kind="ExternalInput").ap()
    yout = nc.dram_tensor("yout", [B, C, H * W], F16, kind="ExternalOutput").ap()

    with tile.TileContext(nc) as tc:
        with (
            tc.tile_pool(name="wpool", bufs=1) as wpool,
            tc.tile_pool(name="xpool", bufs=2) as xpool,
            tc.tile_pool(name="mpool", bufs=1) as mpool,
            tc.tile_pool(name="pspool", bufs=8, space="PSUM") as pspool,
            tc.tile_pool(name="upool", bufs=4) as upool,
            tc.tile_pool(name="opool", bufs=3) as opool,
        ):
            w1p2 = wpool.tile([128, 3 * 128], F16)
            w1p1 = wpool.tile([128, 128], F16)
            w1s = wpool.tile([128, 128], F16)   # content in parts 64-127
            w2p = wpool.tile([128, 3 * 64], F16)
            w2s = wpool.tile([128, 3 * 64], F16)  # content in parts 64-127
            b1t = wpool.tile([128, 1], F32)
            b2t = wpool.tile([64, 1], F32)
            # w1p2 + b1t first (needed by group 0); conv2 weights last
            nc.sync.dma_start(w1p2[:, :], w1p2_d[:, :])
            nc.sync.dma_start(b1t[:, :], b1_d[:, :])
            nc.sync.dma_start(w1p1[:, :], w1p1_d[:, :])
            nc.sync.dma_start(w1s[64:128, :], w1s_d[:, :])
            nc.sync.dma_start(w2p[:, :], w2p_d[:, :])
            nc.sync.dma_start(w2s[64:128, :], w2s_d[:, :])
            nc.sync.dma_start(b2t[:, :], b2_d[:, :])

            def make_x_tiles(img):
                t1 = xpool.tile([128, FLAT], F16, tag="t1", name=f"t1_{img}")
                t2 = xpool.tile([128, FLAT], F16, tag="t2", name=f"t2_{img}")
                return t1, t2

            def load_t2(eng, t2, img, bands):
                src = xin[img]
                for r0, r1 in bands:
                    a, b = r0 * Wp, r1 * Wp
                    eng.dma_start(t2[0:64, a:b], src[:, a:b])
                    bb = min(b, FLAT - 1)
                    eng.dma_start(t2[64:128, a:bb], src[:, a + 1:bb + 1])

            def load_t1(eng, t1, img, bands):
                src = xin[img]
                for r0, r1 in bands:
                    a, b = r0 * Wp, r1 * Wp
                    eng.dma_start(t1[0:64, a:b], src[:, a:b])
                    bb = min(b, FLAT - Wp)
                    if bb > a:
                        eng.dma_start(t1[64:128, a:bb], src[:, a + Wp:bb + Wp])

            # image 0: tiny first band leads each queue so its transfer isn't
            # stuck behind the bulk bands in the shared DMA engine pool; the
            # rest streams progressively, T2 and T1 on separate queues.
            cur = make_x_tiles(0)
            # first T2 rows on gpsimd (no ACT_TABLE_LOAD ahead of it there)
            load_t2(nc.gpsimd, cur[1], 0, [(0, 6)])
            load_t1(nc.gpsimd, cur[0], 0, [(0, 6)])
            bands0 = [(6, 14), (14, 24), (24, 36), (36, 52), (52, 72),
                      (72, 92), (92, Hp)]
            load_t2(nc.scalar, cur[1], 0, bands0)
            load_t1(nc.gpsimd, cur[0], 0, bands0)

            for img in range(B):
                t1f, t2f = cur
                t1 = t1f[:, :].rearrange("p (h w) -> p h w", w=Wp)
                t2 = t2f[:, :].rearrange("p (h w) -> p h w", w=Wp)

                # ---- mid plane M1 = [M ; M>>1], written by ACT/DVE ----
                mt = mpool.tile([128, FLAT], F16, tag="mt", name=f"mt_{img}")
                mr = mt[:, :].rearrange("p (h w) -> p h w", w=Wp)
                nc.gpsimd.memset(mr[0:64, 0, :], 0.0)
                nc.gpsimd.memset(mr[0:64, Hp - 1, :], 0.0)
                nc.gpsimd.memset(mr[64:128, H, :], 0.0)
                nc.gpsimd.memset(mr[:, :, 0], 0.0)
                nc.gpsimd.memset(mr[:, :, Wp - 1], 0.0)

                # ---- conv1 (5 passes/chunk) + bn1 + relu -> M1 ----
                # pass k: 0-2 = taps (k,0)+(k,1) from T2; 3 = (0,2)+(1,2)
                # from T1; 4 = (2,2) from T1 upper. Palindrome order across
                # groups so consecutive groups share the boundary stationary.
                def c1_pass(k, j, g0, pss, start, stop):
                    h0 = (g0 + j) * R
                    if k < 3:
                        nc.tensor.matmul(
                            pss[j][:, :],
                            lhsT=w1p2[:, k * 128:(k + 1) * 128],
                            rhs=t2[0:128, h0 + k:h0 + k + R, 0:W],
                            start=start, stop=stop)
                    elif k == 3:
                        nc.tensor.matmul(
                            pss[j][:, :], lhsT=w1p1[:, :],
                            rhs=t1[0:128, h0:h0 + R, 2:2 + W],
                            start=start, stop=stop)
                    else:
                        nc.tensor.matmul(
                            pss[j][:, :], lhsT=w1s[64:128, :],
                            rhs=t1[64:128, h0 + 1:h0 + 1 + R, 2:2 + W],
                            start=start, stop=stop)

                for gi in range(ngroups):
                    g0 = gi * G
                    pss = [pspool.tile([128, N], F32, tag="ps",
                                       name=f"ps1_{img}_{g0}_{j}")
                           for j in range(G)]
                    order = range(5) if gi % 2 == 0 else range(4, -1, -1)
                    for ki, k in enumerate(order):
                        for j in range(G):
                            c1_pass(k, j, g0, pss, ki == 0, ki == 4)
                    for j in range(G):
                        h0 = (g0 + j) * R
                        ps1 = pss[j]
                        p1lo = ps1[0:64, :].rearrange("p (h w) -> p h w", w=W)
                        p1hi = ps1[64:128, :].rearrange("p (h w) -> p h w", w=W)
                        # M half on ACT, M>>1 half on DVE (split so neither
                        # engine outruns the 5-pass PE group)
                        nc.scalar.activation(
                            mr[0:64, h0 + 1:h0 + 1 + R, 1:W + 1],
                            p1lo, AF.Relu, bias=b1t[0:64, 0:1])
                        nc.vector.tensor_scalar(
                            mr[64:128, h0:h0 + R, 1:W + 1],
                            p1hi, b1t[64:128, 0:1], 0.0, ALU.add, ALU.max)
                    # prefetch next image in bands so its early rows aren't
                    # stuck behind this image's full planes in the DMA pool
                    if img + 1 < B:
                        if gi == 1:
                            nxt = make_x_tiles(img + 1)
                            load_t2(nc.scalar, nxt[1], img + 1, [(0, 38)])
                        elif gi == 3:
                            load_t2(nc.scalar, nxt[1], img + 1, [(38, 76)])
                            load_t1(nc.gpsimd, nxt[0], img + 1, [(0, 38)])
                        elif gi == 5:
                            load_t2(nc.scalar, nxt[1], img + 1, [(76, Hp)])
                            load_t1(nc.gpsimd, nxt[0], img + 1, [(38, 76)])

                if img + 1 < B:
                    load_t1(nc.gpsimd, nxt[0], img + 1, [(76, Hp)])
                    cur = nxt

                # ---- conv2 (6 passes/chunk) + bn2 + residual + relu ----
                def c2_pass(k, j, g0, pss, start, stop):
                    h0 = (g0 + j) * R
                    if k < 3:
                        nc.tensor.matmul(
                            pss[j][:, :],
                            lhsT=w2p[:, k * 64:(k + 1) * 64],
                            rhs=mr[0:128, h0:h0 + R, k:k + W],
                            start=start, stop=stop)
                    else:
                        dw = k - 3
                        nc.tensor.matmul(
                            pss[j][:, :],
                            lhsT=w2s[64:128, dw * 64:(dw + 1) * 64],
                            rhs=mr[64:128, h0 + 1:h0 + 1 + R, dw:dw + W],
                            start=start, stop=stop)

                for gi in range(ngroups):
                    g0 = gi * G
                    pss = [pspool.tile([64, N], F32, tag="ps",
                                       name=f"ps2_{img}_{g0}_{j}")
                           for j in range(G)]
                    order = range(6) if gi % 2 == 0 else range(5, -1, -1)
                    for ki, k in enumerate(order):
                        for j in range(G):
                            c2_pass(k, j, g0, pss, ki == 0, ki == 5)
                    o = opool.tile([64, G * N], F16, tag="o",
                                   name=f"o_{img}_{g0}")
                    for j in range(G):
                        h0 = (g0 + j) * R
                        u = upool.tile([64, N], F32, tag="u",
                                       name=f"u_{img}_{g0}_{j}")
                        nc.vector.tensor_add(
                            u[:, :].rearrange("p (h w) -> p h w", w=W),
                            pss[j][:, :].rearrange("p (h w) -> p h w", w=W),
                            t1[0:64, h0 + 1:h0 + 1 + R, 1:W + 1])
                        nc.vector.tensor_scalar(
                            o[:, j * N:(j + 1) * N], u[:, :],
                            b2t[:, 0:1], 0.0, ALU.add, ALU.max)
                    nc.sync.dma_start(
                        yout[img][:, g0 * N:(g0 + G) * N], o[:, :])
    nc.compile()
    return nc


def prepare_weights(w1, w2, alpha, bn1_gamma, bn1_beta, bn1_mean, bn1_var,
                    bn2_gamma, bn2_beta, bn2_mean, bn2_var):
    w1e = np.einsum('e,eoihw->oihw', alpha.astype(np.float64),
                    w1.astype(np.float64))
    w2e = np.einsum('e,eoihw->oihw', alpha.astype(np.float64),
                    w2.astype(np.float64))
    s1 = bn1_gamma / np.sqrt(bn1_var + EPS)
    b1 = bn1_beta - bn1_mean * s1
    s2 = bn2_gamma / np.sqrt(bn2_var + EPS)
    b2 = bn2_beta - bn2_mean * s2
    w1e = (w1e * s1[:, None, None, None]).astype(np.float16)  # fold bn1 scale
    w2e = (w2e * s2[:, None, None, None]).astype(np.float16)  # fold bn2 scale

    w1p2 = np.zeros((128, 3 * 128), np.float16)
    w1p1 = np.zeros((128, 128), np.float16)
    w1s = np.zeros((64, 128), np.float16)
    w2p = np.zeros((128, 3 * 64), np.float16)
    w2s = np.zeros((64, 3 * 64), np.float16)
    for d in range(3):
        for half in (0, 1):   # K half: T2 lower = tap (d,0), upper = (d,1)
            blk = w1e[:, :, d, half].T
            w1p2[half * 64:(half + 1) * 64, d * 128:d * 128 + 64] = blk
            w1p2[half * 64:(half + 1) * 64, d * 128 + 64:d * 128 + 128] = blk
    for half, dh in ((0, 0), (1, 1)):   # T1 lower = (0,2), upper = (1,2)
        blk = w1e[:, :, dh, 2].T
        w1p1[half * 64:(half + 1) * 64, 0:64] = blk
        w1p1[half * 64:(half + 1) * 64, 64:128] = blk
    w1s[:, 0:64] = w1e[:, :, 2, 2].T
    w1s[:, 64:128] = w1e[:, :, 2, 2].T
    for dw in range(3):
        for dh in (0, 1):
            w2p[dh * 64:(dh + 1) * 64, dw * 64:(dw + 1) * 64] = w2e[:, :, dh, dw].T
        w2s[:, dw * 64:(dw + 1) * 64] = w2e[:, :, 2, dw].T
    b1v = np.tile(b1.astype(np.float32), 2).reshape(128, 1)
    b2v = b2.astype(np.float32).reshape(64, 1)
    return {"w1p2": w1p2, "w1p1": w1p1, "w1s": w1s, "w2p": w2p, "w2s": w2s,
            "b1": b1v, "b2": b2v}


def prepare_x(x):
    """Zero-pad to [B, C, 114, 114] fp16, flattened per image."""
    B, C_, H, W = x.shape
    xp = np.zeros((B, C_, H + 2, W + 2), np.float16)
    xp[:, :, 1:H + 1, 1:W + 1] = x
    return np.ascontiguousarray(xp.reshape(B, C_, (H + 2) * (W + 2)))


def make_in_maps(x, wd):
    B_total = x.shape[0]
    Bc = B_total // N_CORES
    xp = prepare_x(np.asarray(x, np.float32))
    return [dict(wd, xin=np.ascontiguousarray(xp[cid * Bc:(cid + 1) * Bc]))
            for cid in range(N_CORES)]


_NC_CACHE = {}


def kernel(x, w1, w2, alpha,
           bn1_gamma, bn1_beta, bn1_mean, bn1_var,
           bn2_gamma, bn2_beta, bn2_mean, bn2_var):
    x = np.asarray(x, dtype=np.float32)
    B_total, _, H, W = x.shape
    Bc = B_total // N_CORES
    wd = prepare_weights(
        np.asarray(w1, np.float32), np.asarray(w2, np.float32),
        np.asarray(alpha, np.float32),
        np.asarray(bn1_gamma, np.float32), np.asarray(bn1_beta, np.float32),
        np.asarray(bn1_mean, np.float32), np.asarray(bn1_var, np.float32),
        np.asarray(bn2_gamma, np.float32), np.asarray(bn2_beta, np.float32),
        np.asarray(bn2_mean, np.float32), np.asarray(bn2_var, np.float32))

    key = (Bc, H, W)
    if key not in _NC_CACHE:
        _NC_CACHE[key] = build_nc(Bc, H, W)
    nc = _NC_CACHE[key]

    in_maps = make_in_maps(x, wd)
    res = run_bass_kernel_spmd(nc, in_maps, core_ids=list(range(N_CORES)))
    out = np.concatenate([res.results[cid]["yout"] for cid in range(N_CORES)],
                         axis=0)
    return out.reshape(B_total, 64, H, W).astype(np.float32)
